# revision 23
# baseline (speedup 1.0000x reference)
"""Trainium2 Bass kernel for CustomMultiHeadAttention (RoPE + causal MHA).

Sharding: 8 cores = 2 batches x 4 head-groups (4 heads each).
v1 design (all-bf16 inputs, PE-warm scheduling):
  - qk/v projections in bf16 (1 cyc/row, FWL weight loads)
  - scores: pair-fused PSUM tile [128, 2, 512]; the two K=64 score MMs
    target row groups 0/64 and run concurrently in the PE array
  - exp: ONE fused ACT instruction per k-tile covering both heads
    ([128, 2, nv] strided AP); diag blocks masked by gpsimd mult
  - ctx: V_aug (64 v-cols + 64 ones-cols) matmul accumulated over k-tiles;
    ones columns give the softmax denominator replicated on psum rows 64+
  - normalize: ACT Ln -> Exp(-x) reciprocal (both in the
    natural_log_exp_and_others table set => zero table switches), PE
    broadcast matmul to [128,512], DVE multiply
  - emission interleaves next-chunk projection/out-proj matmuls into the
    ACT-bound attention stream so the PE never idles > ~1us (HAM stays
    at K=8/8, 2.4 GHz)
Host: sums the 4 head-group partials per batch (f32), adds bo.
"""

import os
import sys

for _p in ("/opt/trn_rl_repo", "/root/.axon_site/_ro/trn_rl_repo"):
    if os.path.isdir(_p) and _p not in sys.path:
        sys.path.insert(0, _p)

import numpy as np
import ml_dtypes

import concourse.bass as bass
import concourse.bacc as bacc
import concourse.mybir as mybir
import concourse.tile as tile
from concourse.bass_utils import run_bass_kernel_spmd

F32 = mybir.dt.float32
BF16 = mybir.dt.bfloat16
AF = mybir.ActivationFunctionType
ALU = mybir.AluOpType

NUM_HEADS = 16
HD = 64
D = NUM_HEADS * HD  # 1024
B = 2
S = 2048
NCORES = 8
HPC = 4            # heads per core
JC = HPC * HD      # 256 per-core projection width
P = 128
NST = S // P       # 16 seq tiles
NQC = S // 512     # 4 q-chunks
NDT = D // P       # 8 k-tiles over d_model
SCALE = 1.0 / np.sqrt(HD)


def _patch_act_tables(arch):
    """Restrict exp/ln to the combined natural_log_exp_and_others set so
    the table-load pass never alternates between sets (1.28us per load)."""
    from concourse.hw_specs import get_activation_tables
    tabs = get_activation_tables(arch)
    for name, fns in tabs.items():
        if name == "natural_log_exp_and_others":
            continue
        fns.discard(AF.Exp)
        fns.discard(AF.Ln)


def build_core(tc, io):
    nc = tc.nc
    xT_d, wq_d, wk_d, wv_d, wo_d = io["xT"], io["wq"], io["wk"], io["wv"], io["wo"]
    sin_d, cos_d, mask_d, out_d = io["sin"], io["cos"], io["mask2"], io["out"]

    import contextlib
    with contextlib.ExitStack() as ctx:
        cpool = ctx.enter_context(tc.tile_pool(name="const", bufs=1))
        epool = ctx.enter_context(tc.tile_pool(name="expt", bufs=4))
        tpool = ctx.enter_context(tc.tile_pool(name="tmps", bufs=4))
        dpool = ctx.enter_context(tc.tile_pool(name="dens", bufs=2))
        opool = ctx.enter_context(tc.tile_pool(name="ostg", bufs=4))
        ps_sc = ctx.enter_context(tc.tile_pool(name="ps_sc", bufs=2, space="PSUM"))
        ps_ctx = ctx.enter_context(tc.tile_pool(name="ps_cx", bufs=1, space="PSUM"))
        ps_pr = ctx.enter_context(tc.tile_pool(name="ps_pr", bufs=2, space="PSUM"))

        # ---- persistent SBUF tensors ----
        xT = cpool.tile([P, NDT, S], BF16, tag="xT")
        wq = cpool.tile([P, NDT, JC], BF16, tag="wq")
        wk = cpool.tile([P, NDT, JC], BF16, tag="wk")
        wv = cpool.tile([P, NDT, JC], BF16, tag="wv")
        wo = cpool.tile([P, 2, D], BF16, tag="wo")
        sinf = cpool.tile([P, S], BF16, tag="sinf")
        cosf = cpool.tile([P, S], BF16, tag="cosf")
        mask2 = cpool.tile([P, 2, P], BF16, tag="mask2")
        QT = cpool.tile([P, 2, S], BF16, tag="QT")
        KT = cpool.tile([P, 2, S], BF16, tag="KT")
        QTc = cpool.tile([P, 2, S], BF16, tag="QTc")
        KTc = cpool.tile([P, 2, S], BF16, tag="KTc")
        V = cpool.tile([P, NST, HPC, 2 * HD], BF16, tag="V")
        ctxA = cpool.tile([P, S], BF16, tag="ctxA")     # heads 0,1
        ctxB = cpool.tile([P, S], BF16, tag="ctxB")     # heads 2,3

        # ---- initial DMAs. All DRAM tensors are host-packed partition-major
        # ([128, ...] with 2-32KB contiguous runs per partition) so each
        # transfer is 128 large descriptors instead of 1024+ small ones.
        # Interleave wq k-slices with xT k-slices so the first projection's
        # dt_i loop starts ~1us in.
        for dt_i in range(NDT):
            nc.sync.dma_start(wq[:, dt_i, :],
                              wq_d[:, dt_i * JC:(dt_i + 1) * JC])
            nc.sync.dma_start(xT[:, dt_i, :],
                              xT_d[:, dt_i * S:(dt_i + 1) * S])
        nc.sync.dma_start(wk[:], wk_d.rearrange("p (t j) -> p t j", j=JC))
        nc.sync.dma_start(wv[:], wv_d.rearrange("p (t j) -> p t j", j=JC))
        nc.sync.dma_start(sinf[:], sin_d[:])
        nc.sync.dma_start(cosf[:], cos_d[:])
        nc.sync.dma_start(mask2[:], mask_d.rearrange("p (u q) -> p u q", u=2))
        nc.gpsimd.memset(V[:, :, :, HD:], 1.0)   # denominator ones columns

        def qk_proj_units(qc, w_sb, out_sb):
            """Two filler sub-units projecting one 512-col s-chunk of QT or
            KT; the second also applies RoPE and rearranges into QTc/KTc."""
            sl = slice(qc * 512, qc * 512 + 512)
            state = {}

            def mm_jt(pp, jt):
                for dt_i in range(NDT):
                    nc.tensor.matmul(
                        pp[:],
                        lhsT=w_sb[:, dt_i, jt * P:(jt + 1) * P],
                        rhs=xT[:, dt_i, sl],
                        start=(dt_i == 0), stop=(dt_i == NDT - 1),
                    )

            def unit_a():
                pA = ps_pr.tile([P, 512], F32, tag="proj", name="pA")
                mm_jt(pA, 0)
                state["pA"] = pA

            def unit_b():
                pA = state["pA"]
                pB = ps_pr.tile([P, 512], F32, tag="proj", name="pB")
                mm_jt(pB, 1)
                csl, ssl = cosf[:, sl], sinf[:, sl]
                t1 = tpool.tile([P, 512], F32, tag="t1")
                t3 = tpool.tile([P, 512], F32, tag="t2")
                # pA consumed first so its psum slot frees early
                nc.vector.tensor_tensor(t1[:], pA[:], csl, ALU.mult)
                nc.vector.tensor_tensor(t3[:], pA[:], ssl, ALU.mult)
                t2 = tpool.tile([P, 512], F32, tag="t1")
                t4 = tpool.tile([P, 512], F32, tag="t2")
                nc.vector.tensor_tensor(t2[:], pB[:], ssl, ALU.mult)
                # SBUF-only combines go to the idle gpsimd engine
                nc.gpsimd.tensor_tensor(out_sb[:, 0, sl], t1[:], t2[:],
                                        ALU.subtract)
                nc.vector.tensor_tensor(t4[:], pB[:], csl, ALU.mult)
                nc.gpsimd.tensor_tensor(out_sb[:, 1, sl], t3[:], t4[:],
                                        ALU.add)
                dst = QTc if out_sb is QT else KTc
                for h in range(HPC):
                    for half in range(2):
                        nc.sync.dma_start(
                            dst[64 * (h % 2) + 32 * half:
                                64 * (h % 2) + 32 * half + 32, h // 2, sl],
                            out_sb[32 * h:32 * h + 32, half, sl])

            return [unit_a, unit_b]

        def v_proj(st):
            """Project one 128-row seq tile of V (strided dest, ones kept)."""
            pp = ps_pr.tile([P, 512], F32, tag="proj", name="pV")
            for dt_i in range(NDT):
                nc.tensor.matmul(
                    pp[:, :JC],
                    lhsT=xT[:, dt_i, st * P:(st + 1) * P],
                    rhs=wv[:, dt_i, :],
                    start=(dt_i == 0), stop=(dt_i == NDT - 1),
                )
            nc.vector.tensor_copy(
                out=V[:, st, :, 0:HD],
                in_=pp[:, :JC].rearrange("p (h d) -> p h d", h=HPC),
            )

        out_stage = {}

        def out_proj(st, nh):
            """Output projection for one 128-row seq tile, one 512 half;
            the nh=1 half also DMAs the whole [128, 1024] row block out."""
            pp = ps_pr.tile([P, 512], F32, tag="proj", name="pO")
            for jt, csb in enumerate((ctxA, ctxB)):
                nc.tensor.matmul(
                    pp[:],
                    lhsT=csb[:, st * P:(st + 1) * P],
                    rhs=wo[:, jt, nh * 512:nh * 512 + 512],
                    start=(jt == 0), stop=(jt == 1),
                )
            if nh == 0:
                out_stage[st] = opool.tile([P, 1024], BF16, tag="ostage",
                                           name=f"ot{st % 4}")
            ot = out_stage[st]
            nc.vector.tensor_copy(out=ot[:, nh * 512:nh * 512 + 512], in_=pp[:])
            if nh == 1:
                nc.sync.dma_start(out_d[st * P:(st + 1) * P, :], ot[:])
                del out_stage[st]

        # ---------------- attention with interleaved fillers ----------------

        def attention_pair(qc, pair, slots):
            """Causal attention for q-chunk qc, head pair `pair` (heads
            2*pair, 2*pair+1). slots[ki] holds filler closures to emit
            after that k-tile's instructions."""
            n_ki = 4 * qc + 4
            cps = ps_ctx.tile([P, 2, 512], F32, tag="ctx", name=f"cx{qc}_{pair}")
            for ki in range(n_ki):
                diag_r = ki - 4 * qc
                c0 = 128 * diag_r if diag_r >= 0 else 0
                nv = 512 - c0
                qsl = slice(qc * 512 + c0, qc * 512 + 512)
                ksl = slice(ki * P, (ki + 1) * P)
                sc = ps_sc.tile([P, 2, 512], F32, tag="sc", name=f"sc{ki % 2}")
                for hh in range(2):
                    nc.tensor.matmul(
                        sc[:, hh, c0:512],
                        lhsT=KTc[64 * hh:64 * hh + 64, pair, ksl],
                        rhs=QTc[64 * hh:64 * hh + 64, pair, qsl],
                        start=True, stop=True,
                        tile_position=(64 * hh, 0),
                    )
                et = epool.tile([P, 2, 512], BF16, tag="expT")
                nc.scalar.activation(et[:, :, c0:512], sc[:, :, c0:512],
                                     AF.Exp, scale=float(SCALE))
                if diag_r >= 0:
                    nc.gpsimd.tensor_tensor(
                        et[:, :, c0:c0 + P], et[:, :, c0:c0 + P],
                        mask2[:], ALU.mult)
                for hh in range(2):
                    nc.tensor.matmul(
                        cps[:, hh, c0:512],
                        lhsT=V[:, ki, 2 * pair + hh, :],
                        rhs=et[:, hh, c0:512],
                        start=(ki == 0), stop=(ki == n_ki - 1),
                    )
                for f in slots[ki]:
                    f()
            return cps

        def normalize(qc, pair, cps):
            """One DVE copy frees the ctx PSUM slot immediately; the
            reciprocal (ACT Ln/Exp, same table set as the attention Exp),
            gpsimd partition broadcast and DVE multiplies then run entirely
            off the matmul critical path, from SBUF."""
            qsl = slice(qc * 512, qc * 512 + 512)
            csb = dpool.tile([P, 2, 512], BF16, tag="csb")
            nc.vector.tensor_copy(out=csb[:], in_=cps[:])
            t2 = dpool.tile([32, 2, 512], F32, tag="dln")
            r2 = dpool.tile([32, 2, 512], BF16, tag="drec")
            nc.scalar.activation(t2[0:1, :, :], csb[64:65, :, :], AF.Ln)
            nc.scalar.activation(r2[0:1, :, :], t2[0:1, :, :],
                                 AF.Exp, scale=-1.0)
            rb = tpool.tile([64, 2, 512], BF16, tag="rsb")
            for hh in range(2):
                # partition_broadcast can only write at out base partition 0
                nc.gpsimd.partition_broadcast(
                    rb[0:64, hh, :], r2[0:1, hh, :], channels=64)
            dst = ctxA if pair == 0 else ctxB
            for hh in range(2):
                nc.vector.tensor_tensor(
                    dst[64 * hh:64 * hh + 64, qsl],
                    csb[0:64, hh, :], rb[0:64, hh, :], ALU.mult)

        # ---------------- emission schedule ----------------
        for u in qk_proj_units(0, wq, QT):
            u()
        for u in qk_proj_units(0, wk, KT):
            u()
        nc.sync.dma_start(wo[:], wo_d.rearrange("p (t n) -> p t n", n=D))
        for st in range(4):
            v_proj(st)

        for qc in range(NQC):
            fillers = []
            if qc + 1 < NQC:
                fillers += qk_proj_units(qc + 1, wq, QT)
            if qc > 0:
                fillers += [lambda st=st, nh=nh: out_proj(st, nh)
                            for st in range(4 * (qc - 1), 4 * qc)
                            for nh in range(2)]
            if qc + 1 < NQC:
                fillers += qk_proj_units(qc + 1, wk, KT)
                fillers += [lambda st=st: v_proj(st)
                            for st in range(4 * (qc + 1), 4 * qc + 8)]
            # pace fillers evenly across both pairs' k-tiles, keeping the
            # last two k-tiles of each pair filler-free so the normalize
            # copy isn't queued behind filler DVE work
            n_ki = 4 * qc + 4
            slots = [[] for _ in range(2 * n_ki)]
            avoid = {n_ki - 2, n_ki - 1, 2 * n_ki - 2, 2 * n_ki - 1}
            usable = [i for i in range(2 * n_ki) if i not in avoid] or [0]
            for i, f in enumerate(fillers):
                slots[usable[(i * len(usable)) // len(fillers)]].append(f)
            cps0 = attention_pair(qc, 0, slots[:n_ki])
            normalize(qc, 0, cps0)
            cps1 = attention_pair(qc, 1, slots[n_ki:])
            normalize(qc, 1, cps1)
        for st in range(12, 16):
            for nh in range(2):
                out_proj(st, nh)


# ----------------------------------------------------------------------------
# host side
# ----------------------------------------------------------------------------

def _rope_tables():
    pos = np.arange(S, dtype=np.float32)
    inv_freq = np.exp(np.arange(0, HD, 2, dtype=np.float32)
                      * (-np.log(10000.0) / HD)).astype(np.float32)
    ang = pos[:, None] * inv_freq[None, :]          # [S, 32]
    sin = np.sin(ang).astype(np.float32)
    cos = np.cos(ang).astype(np.float32)
    sinf = np.ascontiguousarray(np.tile(sin.T, (HPC, 1)))
    cosf = np.ascontiguousarray(np.tile(cos.T, (HPC, 1)))
    return sinf, cosf


def _half_perm():
    """Column permutation grouping first/second halves of the 4 heads."""
    first = [64 * h + d for h in range(HPC) for d in range(32)]
    second = [64 * h + d for h in range(HPC) for d in range(32, 64)]
    return np.array(first + second, dtype=np.int64)


def build_program():
    nc = bacc.Bacc("TRN2", target_bir_lowering=False, debug=False,
                   num_devices=NCORES)
    _patch_act_tables(nc.m.arch)
    io = {
        "xT": nc.dram_tensor("xT", [P, NDT * S], BF16,
                             kind="ExternalInput").ap(),
        "wq": nc.dram_tensor("wq", [P, NDT * JC], BF16,
                             kind="ExternalInput").ap(),
        "wk": nc.dram_tensor("wk", [P, NDT * JC], BF16,
                             kind="ExternalInput").ap(),
        "wv": nc.dram_tensor("wv", [P, NDT * JC], BF16,
                             kind="ExternalInput").ap(),
        "wo": nc.dram_tensor("wo", [P, 2 * D], BF16,
                             kind="ExternalInput").ap(),
        "sin": nc.dram_tensor("sin", [P, S], BF16, kind="ExternalInput").ap(),
        "cos": nc.dram_tensor("cos", [P, S], BF16, kind="ExternalInput").ap(),
        "mask2": nc.dram_tensor("mask2", [P, 2 * P], BF16,
                                kind="ExternalInput").ap(),
        "out": nc.dram_tensor("out", [S, D], BF16, kind="ExternalOutput").ap(),
    }
    with tile.TileContext(nc) as tc:
        build_core(tc, io)
    nc.compile()
    return nc


def _pack(a, p=P):
    """[T*128, N] row-major -> [128, T*N] partition-major."""
    t = a.shape[0] // p
    return np.ascontiguousarray(
        a.reshape(t, p, -1).transpose(1, 0, 2).reshape(p, -1))


def make_in_maps(x, Wq, Wk, Wv, Wo):
    perm = _half_perm()
    sinf, cosf = _rope_tables()
    mask = np.triu(np.ones((P, P), dtype=np.float32))
    mask2 = np.tile(mask, (1, 2)).astype(ml_dtypes.bfloat16)
    bf = ml_dtypes.bfloat16
    in_maps = []
    for c in range(NCORES):
        b, g = divmod(c, NCORES // B)
        cols = slice(JC * g, JC * (g + 1))
        in_maps.append({
            "xT": _pack(np.ascontiguousarray(x[b].T)).astype(bf),
            "wq": _pack(Wq[:, cols][:, perm]).astype(bf),
            "wk": _pack(Wk[:, cols][:, perm]).astype(bf),
            "wv": _pack(Wv[:, cols]).astype(bf),
            "wo": _pack(np.ascontiguousarray(Wo[cols, :])).astype(bf),
            "sin": sinf.astype(bf), "cos": cosf.astype(bf), "mask2": mask2,
        })
    return in_maps


_CACHED_NC = None


def kernel(x, Wq, bq, Wk, bk, Wv, bv, Wo, bo, **run_kwargs):
    global _CACHED_NC
    x, Wq, bq, Wk, bk, Wv, bv, Wo, bo = (
        np.asarray(a, dtype=np.float32)
        for a in (x, Wq, bq, Wk, bk, Wv, bv, Wo, bo))
    assert not (np.any(bq) or np.any(bk) or np.any(bv)), \
        "nonzero qkv biases not supported by this build"
    if _CACHED_NC is None:
        _CACHED_NC = build_program()
    in_maps = make_in_maps(x, Wq, Wk, Wv, Wo)
    res = run_bass_kernel_spmd(_CACHED_NC, in_maps, list(range(NCORES)),
                               **run_kwargs)
    out = np.zeros((B, S, D), dtype=np.float32)
    for c in range(NCORES):
        b = c // (NCORES // B)
        out[b] += res.results[c]["out"].astype(np.float32)
    out += bo[None, None, :]
    if run_kwargs:
        kernel.last_result = res
    return out
